# revision 32
# baseline (speedup 1.0000x reference)
"""Causal multi-head attention (B=2, L=2048, D=1024, H=16, Dh=64) on 8 TRN2
NeuronCores.

Sharding: data-parallel over B (2 groups of 4 cores), tensor-parallel over H
within a group (4 heads per core). Each core computes QKV projections for its
heads, full causal attention per head (flash-style, scores kept transposed so
no on-chip transposes are needed), and a partial output projection
y_c = sum_h o_h @ Wout_h. The host sums the 4 partials per batch.

Per-core layout choices:
  - x is pre-transposed on the host (xT [D, L]) so the QKV contraction dim D
    lands on SBUF partitions directly.
  - q, k are produced transposed (qT/kT [e, L]) so the scores matmul
    ST = K_h @ Q_h^T contracts over Dh on partitions; softmax runs on ST
    tiles [k=128, q=512] with the reduction (sum over k) folded into the
    P@V matmul via a ones-row appended to V (lhsT [128, 65]; row 64 of the
    PSUM result is the softmax denominator).
  - Projections run in float32r (TF32-class, 1 cycle/row at N>=256);
    the attention matmuls (scores, P@V) run in bf16 with f32 PSUM
    accumulation, which makes their weight loads FWL-fast.
"""

import numpy as np

import concourse.bass as bass
import concourse.mybir as mybir
import concourse.tile as tile
from concourse import bacc
from concourse.bass_utils import run_bass_kernel_spmd

F32 = mybir.dt.float32
F32R = mybir.dt.float32r
BF16 = mybir.dt.bfloat16
EXP = mybir.ActivationFunctionType.Exp
MULT = mybir.AluOpType.mult

B, L, D, H = 2, 2048, 1024, 16
Dh = D // H
NCORES = 8
NH = 4            # heads per core
EL = NH * Dh      # local head dims = 256
P = 128
NQ = 512          # q-chunk width (scores free dim)
QC = L // NQ      # 4 q-chunks
DC = D // P       # 8 contraction chunks for projections
LC = 4            # xT l-chunks for QKV
NL = L // LC      # 512


def build():
    nc = bacc.Bacc("TRN2", target_bir_lowering=False, debug=False,
                   num_devices=NCORES)

    xT = nc.dram_tensor("xT", [D, L], BF16, kind="ExternalInput")
    wq = nc.dram_tensor("wq", [D, EL], BF16, kind="ExternalInput")
    wk = nc.dram_tensor("wk", [D, EL], BF16, kind="ExternalInput")
    wv = nc.dram_tensor("wv", [D, EL], BF16, kind="ExternalInput")
    wout = nc.dram_tensor("wout", [EL, D], BF16, kind="ExternalInput")
    masks = nc.dram_tensor("masks", [P, P], BF16, kind="ExternalInput")
    out = nc.dram_tensor("out", [L, D], F32, kind="ExternalOutput")

    scale = 1.0 / np.sqrt(Dh)

    with tile.TileContext(nc) as tc:
        with (
            tc.tile_pool(name="const", bufs=1) as cpool,
            tc.tile_pool(name="xt", bufs=2) as xpool,
            tc.tile_pool(name="pt", bufs=8) as ptpool,
            tc.tile_pool(name="work", bufs=2) as wpool,
            tc.tile_pool(name="dram", bufs=2, space="DRAM") as dpool,
            tc.tile_pool(name="mm", bufs=2, space="PSUM") as mm_ps,
            tc.tile_pool(name="st", bufs=3, space="PSUM") as st_ps,
            tc.tile_pool(name="pv", bufs=2, space="PSUM") as pv_ps,
            tc.tile_pool(name="bc", bufs=1, space="PSUM") as bc_ps,
        ):
            # ---- persistent SBUF tensors ----
            wq_sb = cpool.tile([P, DC, EL], BF16, tag="wq")
            wk_sb = cpool.tile([P, DC, EL], BF16, tag="wk")
            wv_sb = cpool.tile([P, DC, EL], BF16, tag="wv")
            wout_sb = cpool.tile([P, EL // P, D], BF16, tag="wout")
            mask_sb = cpool.tile([P, P], BF16, tag="mask")
            qT_sb = cpool.tile([P, EL // P, L], BF16, tag="qT")
            kT_sb = cpool.tile([P, EL // P, L], BF16, tag="kT")
            vext_sb = cpool.tile([P, L // P, NH, Dh + 1], BF16, tag="vext")
            oT_sb = cpool.tile([P, EL // P, L], BF16, tag="oT")
            ones_f32 = cpool.tile([P, P], F32, tag="onesf")
            ones_sb = cpool.tile([P, P], F32R, tag="ones")

            # DMA order matters at startup: the first QKV matmul group needs
            # wq + the first xT chunk; everything else can trickle in behind
            xT_r = xT.ap().rearrange("(o p) l -> p o l", p=P)
            nc.sync.dma_start(
                wq_sb[:], wq.ap().rearrange("(o p) e -> p o e", p=P))
            xt0 = xpool.tile([P, DC, NL], BF16, tag="xt", name="xt0")
            nc.sync.dma_start(xt0[:], xT_r[:, :, 0:NL])
            nc.sync.dma_start(
                wk_sb[:], wk.ap().rearrange("(o p) e -> p o e", p=P))
            nc.sync.dma_start(
                wv_sb[:], wv.ap().rearrange("(o p) e -> p o e", p=P))
            nc.sync.dma_start(
                wout_sb[:], wout.ap().rearrange("(o p) d -> p o d", p=P))
            nc.sync.dma_start(mask_sb[:], masks[:, :])

            nc.vector.memset(ones_f32[:], 1.0)
            nc.vector.tensor_copy(out=ones_sb[:], in_=ones_f32[:])
            # ones column of vext (the softmax-denominator row of P@V)
            nc.vector.tensor_copy(
                out=vext_sb[:, :, :, Dh],
                in_=ones_f32[:, 0:L // P * NH].rearrange("p (a b) -> p a b", a=L // P),
            )

            # ---- QKV projections, streaming xT in l-chunks of 512 ----
            for lc in range(LC):
                if lc == 0:
                    xt = xt0
                else:
                    xt = xpool.tile([P, DC, NL], BF16, tag="xt")
                    nc.sync.dma_start(xt[:], xT_r[:, :, lc * NL:(lc + 1) * NL])

                for w_sb, dst in ((wq_sb, qT_sb), (wk_sb, kT_sb)):
                    for ec in range(EL // P):
                        ps = mm_ps.tile([P, NQ], F32, tag="mm",
                                        name=f"qk_{lc}_{ec}")
                        for dc in range(DC):
                            nc.tensor.matmul(
                                ps[:],
                                w_sb[:, dc, ec * P:(ec + 1) * P],
                                xt[:, dc, :],
                                start=(dc == 0), stop=(dc == DC - 1),
                            )
                        nc.any.tensor_copy(
                            out=dst[:, ec, lc * NL:(lc + 1) * NL], in_=ps[:])

                for lt in range(NL // P):
                    lo = lc * (NL // P) + lt
                    ps = mm_ps.tile([P, EL], F32, tag="mm",
                                    name=f"v_{lc}_{lt}")
                    for dc in range(DC):
                        nc.tensor.matmul(
                            ps[:],
                            xt[:, dc, lt * P:(lt + 1) * P],
                            wv_sb[:, dc, :],
                            start=(dc == 0), stop=(dc == DC - 1),
                        )
                    nc.any.tensor_copy(
                        out=vext_sb[:, lo, :, 0:Dh],
                        in_=ps[:].rearrange("p (h e) -> p h e", h=NH),
                    )

            # ---- attention: per (q-chunk, head-pair) ----
            for qc in range(QC):
                nk = 4 * (qc + 1)          # causal k-chunks of 128
                for pair in range(NH // 2):
                    heads = (2 * pair, 2 * pair + 1)
                    pts = {}               # (h, ki) -> pt tile
                    pvs = {}               # h -> accumulating PSUM tile
                    for ki in range(nk):
                        j = ki - 4 * qc    # >=0 on diagonal-crossing tiles
                        for h in heads:
                            hp = (h % 2) * 64
                            ec = h // 2
                            st = st_ps.tile([P, NQ], F32, tag="st",
                                            name=f"st_{qc}_{h}_{ki}")
                            nc.tensor.matmul(
                                st[:],
                                kT_sb[hp:hp + 64, ec, ki * P:(ki + 1) * P],
                                qT_sb[hp:hp + 64, ec, qc * NQ:(qc + 1) * NQ],
                                start=True, stop=True,
                            )
                            pt = ptpool.tile([P, NQ], BF16, tag="pt")
                            if j < 0:
                                nc.scalar.activation(out=pt[:], in_=st[:],
                                                     func=EXP, scale=scale)
                            else:
                                # columns left of the diagonal block are fully
                                # masked; the diagonal 128-block needs the
                                # triangular mask; the rest is unmasked
                                if j > 0:
                                    nc.vector.memset(pt[:, 0:P * j], 0.0)
                                nc.scalar.activation(
                                    out=pt[:, P * j:], in_=st[:, P * j:],
                                    func=EXP, scale=scale)
                                nc.vector.tensor_tensor(
                                    out=pt[:, P * j:P * (j + 1)],
                                    in0=pt[:, P * j:P * (j + 1)],
                                    in1=mask_sb[:, :], op=MULT)
                            pts[(h, ki)] = pt
                        for h in heads:
                            if ki == 0:
                                pvs[h] = pv_ps.tile([Dh + 1, NQ], F32,
                                                    name=f"po_{qc}_{h}",
                                                    tag="pv")
                            nc.tensor.matmul(
                                pvs[h][:],
                                vext_sb[:, ki, h, :],
                                pts[(h, ki)][:],
                                start=(ki == 0), stop=(ki == nk - 1),
                            )

                    for h in heads:
                        hp = (h % 2) * 64
                        ec = h // 2
                        po = pvs[h]
                        # evacuate PSUM immediately so the next pair's P@V
                        # can claim the slot; the norm chain runs SBUF-side
                        ot_un = wpool.tile([64, NQ], F32, tag="otun")
                        nc.vector.tensor_copy(out=ot_un[:], in_=po[0:64, :])
                        rsum = wpool.tile([P, NQ], F32, tag="rsum")
                        nc.vector.tensor_copy(out=rsum[64:65, :],
                                              in_=po[64:65, :])
                        # reshape the [1,512] rsum row to [64,8] via a DRAM
                        # bounce so the exact reciprocal uses 64 DVE lanes
                        rr_f = wpool.tile([P, NQ], F32, tag="rrf")
                        dr1 = dpool.tile([NQ], F32, name=f"dr1_{qc}_{h}",
                                         tag="dr1")
                        nc.sync.dma_start(
                            dr1[:].rearrange("(a b) -> a b", a=1),
                            rsum[64:65, :])
                        r64 = wpool.tile([64, NQ // 64], F32, tag="r64")
                        nc.sync.dma_start(
                            r64[:], dr1[:].rearrange("(a b) -> a b", b=NQ // 64))
                        nc.vector.reciprocal(r64[:], r64[:])
                        dr2 = dpool.tile([NQ], F32, name=f"dr2_{qc}_{h}",
                                         tag="dr2")
                        nc.sync.dma_start(
                            dr2[:].rearrange("(a b) -> a b", b=NQ // 64), r64[:])
                        nc.sync.dma_start(
                            rr_f[64:65, :],
                            dr2[:].rearrange("(a b) -> a b", a=1))
                        rr = wpool.tile([P, NQ], F32R, tag="rr")
                        nc.vector.tensor_copy(out=rr[64:65, :],
                                              in_=rr_f[64:65, :])
                        ps_bc = bc_ps.tile([64, NQ], F32, tag="bc",
                                           name=f"bc_{qc}_{h}")
                        nc.tensor.matmul(ps_bc[:], ones_sb[64:65, 0:64],
                                         rr[64:65, :], start=True, stop=True)
                        rs_sb = wpool.tile([64, NQ], F32, tag="rs")
                        nc.vector.tensor_copy(out=rs_sb[:], in_=ps_bc[:])
                        tmp = wpool.tile([64, NQ], BF16, tag="tmp")
                        nc.vector.tensor_tensor(out=tmp[:], in0=ot_un[:],
                                                in1=rs_sb[:], op=MULT)
                        nc.sync.dma_start(
                            oT_sb[hp:hp + 64, ec, qc * NQ:(qc + 1) * NQ],
                            tmp[:])

            # ---- output projection: y = oT^T @ wout (partial over heads) ----
            for lt in range(L // P):
                for do in range(D // NQ):
                    ps = mm_ps.tile([P, NQ], F32, tag="mm",
                                    name=f"y_{lt}_{do}")
                    for ec in range(EL // P):
                        nc.tensor.matmul(
                            ps[:],
                            oT_sb[:, ec, lt * P:(lt + 1) * P],
                            wout_sb[:, ec, do * NQ:(do + 1) * NQ],
                            start=(ec == 0), stop=(ec == EL // P - 1),
                        )
                    y_sb = wpool.tile([P, NQ], F32, tag="y")
                    nc.any.tensor_copy(out=y_sb[:], in_=ps[:])
                    nc.sync.dma_start(
                        out.ap()[lt * P:(lt + 1) * P, do * NQ:(do + 1) * NQ],
                        y_sb[:])

    nc.compile()
    return nc


def _host_masks():
    k = np.arange(P)[:, None]
    q = np.arange(P)[None, :]
    return (k <= q).astype(np.float32)


def _shard(x, Wq, Wk, Wv, Wout):
    import ml_dtypes
    bf16 = ml_dtypes.bfloat16
    masks = _host_masks()
    in_maps = []
    for c in range(NCORES):
        b, g = c // NH, c % NH
        hs = slice(g * NH, (g + 1) * NH)
        in_maps.append({
            "xT": np.ascontiguousarray(x[b].T).astype(bf16),
            "wq": np.ascontiguousarray(Wq[:, hs, :].reshape(D, EL)).astype(bf16),
            "wk": np.ascontiguousarray(Wk[:, hs, :].reshape(D, EL)).astype(bf16),
            "wv": np.ascontiguousarray(Wv[:, hs, :].reshape(D, EL)).astype(bf16),
            "wout": np.ascontiguousarray(Wout[hs].reshape(EL, D)).astype(bf16),
            "masks": masks.astype(bf16),
        })
    return in_maps


_NC_CACHE = None


def _get_nc():
    global _NC_CACHE
    if _NC_CACHE is None:
        _NC_CACHE = build()
    return _NC_CACHE


def run(x, Wq, Wk, Wv, Wout, trace=False):
    nc = _get_nc()
    in_maps = _shard(np.asarray(x), np.asarray(Wq), np.asarray(Wk),
                     np.asarray(Wv), np.asarray(Wout))
    res = run_bass_kernel_spmd(nc, in_maps, core_ids=list(range(NCORES)),
                               trace=trace)
    parts = [res.results[c]["out"] for c in range(NCORES)]
    full = np.stack([
        parts[0] + parts[1] + parts[2] + parts[3],
        parts[4] + parts[5] + parts[6] + parts[7],
    ]).astype(np.float32)
    return full, res


def kernel(x, Wq, Wk, Wv, Wout):
    full, _ = run(x, Wq, Wk, Wv, Wout, trace=False)
    return full


# revision 33
# speedup vs baseline: 1.0349x; 1.0349x over previous
"""Causal multi-head attention (B=2, L=2048, D=1024, H=16, Dh=64) on 8 TRN2
NeuronCores.

Sharding: data-parallel over B (2 groups of 4 cores), tensor-parallel over H
within a group (4 heads per core). Each core computes QKV projections for its
heads, full causal attention per head (flash-style, scores kept transposed so
no on-chip transposes are needed), and a partial output projection
y_c = sum_h o_h @ Wout_h. The host sums the 4 partials per batch.

Per-core layout choices:
  - x is pre-transposed on the host (xT [D, L]) so the QKV contraction dim D
    lands on SBUF partitions directly.
  - q, k are produced transposed (qT/kT [e, L]) so the scores matmul
    ST = K_h @ Q_h^T contracts over Dh on partitions; softmax runs on ST
    tiles [k=128, q=512] with the reduction (sum over k) folded into the
    P@V matmul via a ones-row appended to V (lhsT [128, 65]; row 64 of the
    PSUM result is the softmax denominator).
  - Projections run in float32r (TF32-class, 1 cycle/row at N>=256);
    the attention matmuls (scores, P@V) run in bf16 with f32 PSUM
    accumulation, which makes their weight loads FWL-fast.
"""

import numpy as np

import concourse.bass as bass
import concourse.mybir as mybir
import concourse.tile as tile
from concourse import bacc
from concourse.bass_utils import run_bass_kernel_spmd

F32 = mybir.dt.float32
F32R = mybir.dt.float32r
BF16 = mybir.dt.bfloat16
EXP = mybir.ActivationFunctionType.Exp
MULT = mybir.AluOpType.mult

B, L, D, H = 2, 2048, 1024, 16
Dh = D // H
NCORES = 8
NH = 4            # heads per core
EL = NH * Dh      # local head dims = 256
P = 128
NQ = 512          # q-chunk width (scores free dim)
QC = L // NQ      # 4 q-chunks
DC = D // P       # 8 contraction chunks for projections
LC = 4            # xT l-chunks for QKV
NL = L // LC      # 512


def build():
    nc = bacc.Bacc("TRN2", target_bir_lowering=False, debug=False,
                   num_devices=NCORES)

    xT = nc.dram_tensor("xT", [D, L], BF16, kind="ExternalInput")
    wq = nc.dram_tensor("wq", [D, EL], BF16, kind="ExternalInput")
    wk = nc.dram_tensor("wk", [D, EL], BF16, kind="ExternalInput")
    wv = nc.dram_tensor("wv", [D, EL], BF16, kind="ExternalInput")
    wout = nc.dram_tensor("wout", [EL, D], BF16, kind="ExternalInput")
    masks = nc.dram_tensor("masks", [P, P], BF16, kind="ExternalInput")
    out = nc.dram_tensor("out", [L, D], F32, kind="ExternalOutput")

    scale = 1.0 / np.sqrt(Dh)

    with tile.TileContext(nc) as tc:
        with (
            tc.tile_pool(name="const", bufs=1) as cpool,
            tc.tile_pool(name="xt", bufs=2) as xpool,
            tc.tile_pool(name="pt", bufs=8) as ptpool,
            tc.tile_pool(name="work", bufs=2) as wpool,
            tc.tile_pool(name="dram", bufs=2, space="DRAM") as dpool,
            tc.tile_pool(name="mm", bufs=2, space="PSUM") as mm_ps,
            tc.tile_pool(name="st", bufs=3, space="PSUM") as st_ps,
            tc.tile_pool(name="pv", bufs=2, space="PSUM") as pv_ps,
            tc.tile_pool(name="bc", bufs=1, space="PSUM") as bc_ps,
        ):
            # ---- persistent SBUF tensors ----
            wq_sb = cpool.tile([P, DC, EL], BF16, tag="wq")
            wk_sb = cpool.tile([P, DC, EL], BF16, tag="wk")
            wv_sb = cpool.tile([P, DC, EL], BF16, tag="wv")
            wout_sb = cpool.tile([P, EL // P, D], BF16, tag="wout")
            mask_sb = cpool.tile([P, P], BF16, tag="mask")
            qT_sb = cpool.tile([P, EL // P, L], BF16, tag="qT")
            kT_sb = cpool.tile([P, EL // P, L], BF16, tag="kT")
            vext_sb = cpool.tile([P, L // P, NH, Dh + 1], BF16, tag="vext")
            oT_sb = cpool.tile([P, EL // P, L], BF16, tag="oT")
            ones_f32 = cpool.tile([P, P], F32, tag="onesf")
            ones_sb = cpool.tile([P, P], F32R, tag="ones")

            # DMA order matters at startup: the first QKV matmul group needs
            # wq + the first xT chunk; everything else can trickle in behind
            xT_r = xT.ap().rearrange("(o p) l -> p o l", p=P)
            wq_r = wq.ap().rearrange("(o p) e -> p o e", p=P)
            xt0 = xpool.tile([P, DC, NL], BF16, tag="xt", name="xt0")
            # split the first loads across several DMA queues so the first
            # matmul group isn't gated on two single-queue transfers
            for dc in range(0, DC, 2):
                nc.sync.dma_start(wq_sb[:, dc:dc + 2, :], wq_r[:, dc:dc + 2, :])
                nc.sync.dma_start(xt0[:, dc:dc + 2, :], xT_r[:, dc:dc + 2, 0:NL])
            nc.sync.dma_start(
                wk_sb[:], wk.ap().rearrange("(o p) e -> p o e", p=P))
            nc.sync.dma_start(
                wv_sb[:], wv.ap().rearrange("(o p) e -> p o e", p=P))
            nc.sync.dma_start(
                wout_sb[:], wout.ap().rearrange("(o p) d -> p o d", p=P))
            nc.sync.dma_start(mask_sb[:], masks[:, :])

            nc.vector.memset(ones_f32[:], 1.0)
            nc.vector.tensor_copy(out=ones_sb[:], in_=ones_f32[:])
            # ones column of vext (the softmax-denominator row of P@V)
            nc.vector.tensor_copy(
                out=vext_sb[:, :, :, Dh],
                in_=ones_f32[:, 0:L // P * NH].rearrange("p (a b) -> p a b", a=L // P),
            )

            # ---- QKV projections, streaming xT in l-chunks of 512 ----
            for lc in range(LC):
                if lc == 0:
                    xt = xt0
                else:
                    xt = xpool.tile([P, DC, NL], BF16, tag="xt")
                    nc.sync.dma_start(xt[:], xT_r[:, :, lc * NL:(lc + 1) * NL])

                for w_sb, dst in ((wq_sb, qT_sb), (wk_sb, kT_sb)):
                    for ec in range(EL // P):
                        ps = mm_ps.tile([P, NQ], F32, tag="mm",
                                        name=f"qk_{lc}_{ec}")
                        for dc in range(DC):
                            nc.tensor.matmul(
                                ps[:],
                                w_sb[:, dc, ec * P:(ec + 1) * P],
                                xt[:, dc, :],
                                start=(dc == 0), stop=(dc == DC - 1),
                            )
                        nc.any.tensor_copy(
                            out=dst[:, ec, lc * NL:(lc + 1) * NL], in_=ps[:])

                for lt in range(NL // P):
                    lo = lc * (NL // P) + lt
                    ps = mm_ps.tile([P, EL], F32, tag="mm",
                                    name=f"v_{lc}_{lt}")
                    for dc in range(DC):
                        nc.tensor.matmul(
                            ps[:],
                            xt[:, dc, lt * P:(lt + 1) * P],
                            wv_sb[:, dc, :],
                            start=(dc == 0), stop=(dc == DC - 1),
                        )
                    nc.any.tensor_copy(
                        out=vext_sb[:, lo, :, 0:Dh],
                        in_=ps[:].rearrange("p (h e) -> p h e", h=NH),
                    )

            # ---- attention: per (q-chunk, head-pair) ----
            for qc in range(QC):
                nk = 4 * (qc + 1)          # causal k-chunks of 128
                for pair in range(NH // 2):
                    heads = (2 * pair, 2 * pair + 1)
                    pts = {}               # (h, ki) -> pt tile
                    pvs = {}               # h -> accumulating PSUM tile
                    for ki in range(nk):
                        j = ki - 4 * qc    # >=0 on diagonal-crossing tiles
                        for h in heads:
                            hp = (h % 2) * 64
                            ec = h // 2
                            st = st_ps.tile([P, NQ], F32, tag="st",
                                            name=f"st_{qc}_{h}_{ki}")
                            nc.tensor.matmul(
                                st[:],
                                kT_sb[hp:hp + 64, ec, ki * P:(ki + 1) * P],
                                qT_sb[hp:hp + 64, ec, qc * NQ:(qc + 1) * NQ],
                                start=True, stop=True,
                            )
                            pt = ptpool.tile([P, NQ], BF16, tag="pt")
                            if j < 0:
                                nc.scalar.activation(out=pt[:], in_=st[:],
                                                     func=EXP, scale=scale)
                            else:
                                # columns left of the diagonal block are fully
                                # masked; the diagonal 128-block needs the
                                # triangular mask; the rest is unmasked
                                if j > 0:
                                    nc.vector.memset(pt[:, 0:P * j], 0.0)
                                nc.scalar.activation(
                                    out=pt[:, P * j:], in_=st[:, P * j:],
                                    func=EXP, scale=scale)
                                nc.vector.tensor_tensor(
                                    out=pt[:, P * j:P * (j + 1)],
                                    in0=pt[:, P * j:P * (j + 1)],
                                    in1=mask_sb[:, :], op=MULT)
                            pts[(h, ki)] = pt
                        for h in heads:
                            if ki == 0:
                                pvs[h] = pv_ps.tile([Dh + 1, NQ], F32,
                                                    name=f"po_{qc}_{h}",
                                                    tag="pv")
                            nc.tensor.matmul(
                                pvs[h][:],
                                vext_sb[:, ki, h, :],
                                pts[(h, ki)][:],
                                start=(ki == 0), stop=(ki == nk - 1),
                            )

                    for h in heads:
                        hp = (h % 2) * 64
                        ec = h // 2
                        po = pvs[h]
                        # evacuate PSUM immediately so the next pair's P@V
                        # can claim the slot; the norm chain runs SBUF-side
                        ot_un = wpool.tile([64, NQ], F32, tag="otun")
                        nc.vector.tensor_copy(out=ot_un[:], in_=po[0:64, :])
                        rsum = wpool.tile([P, NQ], F32, tag="rsum")
                        nc.vector.tensor_copy(out=rsum[64:65, :],
                                              in_=po[64:65, :])
                        # reshape the [1,512] rsum row to [64,8] via a DRAM
                        # bounce so the exact reciprocal uses 64 DVE lanes
                        rr_f = wpool.tile([P, NQ], F32, tag="rrf")
                        dr1 = dpool.tile([NQ], F32, name=f"dr1_{qc}_{h}",
                                         tag="dr1")
                        nc.sync.dma_start(
                            dr1[:].rearrange("(a b) -> a b", a=1),
                            rsum[64:65, :])
                        r64 = wpool.tile([64, NQ // 64], F32, tag="r64")
                        nc.sync.dma_start(
                            r64[:], dr1[:].rearrange("(a b) -> a b", b=NQ // 64))
                        nc.vector.reciprocal(r64[:], r64[:])
                        dr2 = dpool.tile([NQ], F32, name=f"dr2_{qc}_{h}",
                                         tag="dr2")
                        nc.sync.dma_start(
                            dr2[:].rearrange("(a b) -> a b", b=NQ // 64), r64[:])
                        nc.sync.dma_start(
                            rr_f[64:65, :],
                            dr2[:].rearrange("(a b) -> a b", a=1))
                        rr = wpool.tile([P, NQ], F32R, tag="rr")
                        nc.vector.tensor_copy(out=rr[64:65, :],
                                              in_=rr_f[64:65, :])
                        ps_bc = bc_ps.tile([64, NQ], F32, tag="bc",
                                           name=f"bc_{qc}_{h}")
                        nc.tensor.matmul(ps_bc[:], ones_sb[64:65, 0:64],
                                         rr[64:65, :], start=True, stop=True)
                        rs_sb = wpool.tile([64, NQ], F32, tag="rs")
                        nc.vector.tensor_copy(out=rs_sb[:], in_=ps_bc[:])
                        tmp = wpool.tile([64, NQ], BF16, tag="tmp")
                        nc.vector.tensor_tensor(out=tmp[:], in0=ot_un[:],
                                                in1=rs_sb[:], op=MULT)
                        nc.sync.dma_start(
                            oT_sb[hp:hp + 64, ec, qc * NQ:(qc + 1) * NQ],
                            tmp[:])

            # ---- output projection: y = oT^T @ wout (partial over heads) ----
            for lt in range(L // P):
                for do in range(D // NQ):
                    ps = mm_ps.tile([P, NQ], F32, tag="mm",
                                    name=f"y_{lt}_{do}")
                    for ec in range(EL // P):
                        nc.tensor.matmul(
                            ps[:],
                            oT_sb[:, ec, lt * P:(lt + 1) * P],
                            wout_sb[:, ec, do * NQ:(do + 1) * NQ],
                            start=(ec == 0), stop=(ec == EL // P - 1),
                        )
                    y_sb = wpool.tile([P, NQ], F32, tag="y")
                    nc.any.tensor_copy(out=y_sb[:], in_=ps[:])
                    nc.sync.dma_start(
                        out.ap()[lt * P:(lt + 1) * P, do * NQ:(do + 1) * NQ],
                        y_sb[:])

    nc.compile()
    return nc


def _host_masks():
    k = np.arange(P)[:, None]
    q = np.arange(P)[None, :]
    return (k <= q).astype(np.float32)


def _shard(x, Wq, Wk, Wv, Wout):
    import ml_dtypes
    bf16 = ml_dtypes.bfloat16
    masks = _host_masks()
    in_maps = []
    for c in range(NCORES):
        b, g = c // NH, c % NH
        hs = slice(g * NH, (g + 1) * NH)
        in_maps.append({
            "xT": np.ascontiguousarray(x[b].T).astype(bf16),
            "wq": np.ascontiguousarray(Wq[:, hs, :].reshape(D, EL)).astype(bf16),
            "wk": np.ascontiguousarray(Wk[:, hs, :].reshape(D, EL)).astype(bf16),
            "wv": np.ascontiguousarray(Wv[:, hs, :].reshape(D, EL)).astype(bf16),
            "wout": np.ascontiguousarray(Wout[hs].reshape(EL, D)).astype(bf16),
            "masks": masks.astype(bf16),
        })
    return in_maps


_NC_CACHE = None


def _get_nc():
    global _NC_CACHE
    if _NC_CACHE is None:
        _NC_CACHE = build()
    return _NC_CACHE


def run(x, Wq, Wk, Wv, Wout, trace=False):
    nc = _get_nc()
    in_maps = _shard(np.asarray(x), np.asarray(Wq), np.asarray(Wk),
                     np.asarray(Wv), np.asarray(Wout))
    res = run_bass_kernel_spmd(nc, in_maps, core_ids=list(range(NCORES)),
                               trace=trace)
    parts = [res.results[c]["out"] for c in range(NCORES)]
    full = np.stack([
        parts[0] + parts[1] + parts[2] + parts[3],
        parts[4] + parts[5] + parts[6] + parts[7],
    ]).astype(np.float32)
    return full, res


def kernel(x, Wq, Wk, Wv, Wout):
    full, _ = run(x, Wq, Wk, Wv, Wout, trace=False)
    return full
